# revision 9
# baseline (speedup 1.0000x reference)
"""Trainium2 Bass kernel for nn_LogicConvSparseMatrix.

Math: the reference's 15-term weighted logic-op sum collapses to

    out[b,k] = C_ab[k]*A*B + C_a[k]*A + C_b[k]*B + C_1[k]

where A = x[b, ca_k, ha_k+oh, wa_k+ow], B = x[b, cb_k, hb_k+oh, wb_k+ow]
are shifted 126x126 windows.  With alpha = C_b/C_ab, gamma = C_1 -
C_a*C_b/C_ab this factors into

    out = (A + alpha) * (C_ab*B + C_a) + gamma

Per kernel k, three element passes:
  1. affine:  B2 = C_ab*B + C_a           (ScalarE activation or DVE TS)
  2. product: T = (A + alpha) * B2        (DVE scalar_tensor_tensor)
  3. +gamma -> fp32 output staging tile O (ScalarE / DVE TS / GpSimd TT,
     round-robin per group for load balance)

Index pairs are known at build time, so gathers are compile-time SBUF
views of X[p=h, (c,b,w)].  Compute-engine SBUF operands may only start at
partition 0/32/64/96; the relative h-shift between the two windows is
materialized as shifted column copies loaded straight from DRAM.

dtype: x is pre-cast to bf16 on host (halves load bytes; DVE gets 2x
mode).  Intermediates (b2, T) are bf16 only for kernels whose simulated
bf16 error (on batch 0) stays under BF16_ERR_THRESH of the output
absmax; risky kernels (large alpha/gamma amplification) run with fp32
intermediates.  The gamma pass always writes the fp32 staging tile O.

DMA routing (all measured on HW):
  - X loads: 8 c-chunks on the SP HWDGE ring ([H,C,BPC,W] h-major DRAM
    layout -> 8KB-per-partition contiguous descriptors, line rate).
    Compute on early channels starts while later chunks stream.
  - shifted-run/filler/gcol loads: SWDGE (gpsimd queue) under
    tc.high_priority().  Their small 256KB-strided descriptors all hash
    onto a single SDMA engine under HWDGE (measured 23 GB/s); SWDGE
    round-robins descriptors across all 16 engines.  high_priority stops
    the Tile scheduler from parking them behind early compute.
  - stores: SWDGE from the GpSimd queue (issue ~0.7us, transfer async).
    Device output layout [OH, Ksorted, BPC, OW] makes one batched
    same-base group store a run of ~8KB contiguous per-partition
    descriptors (bursts at line rate).  The host inverse-permutes and
    transposes back to [B, K, OH, OW].

No DVE op may use a 2-port perf mode (fp32 copy / even-innermost bf16
tensor_scalar): that locks GpSimd out of SBUF and starves SWDGE
descriptor generation.  All DVE tensor_scalar APs are reshaped to an odd
innermost dim (63) to cap them at 2x_1P.

Sharding: data-parallel over batch, 2 batch items per core, 8 cores.
"""

import numpy as np

B, C, H, W = 16, 64, 128, 128
K = 128
RH = RW = 3
OH, OW = H - RH + 1, W - RW + 1
NCORES = 8
BPC = B // NCORES

GRP = 8  # kernels per store group
NXCHUNK = 8  # X load chunks on the SP ring
CSZ = C // NXCHUNK
BF16_ERR_THRESH = 1.5e-3  # sim bf16 err (rel to absmax) above which k goes fp32
GSPLIT = ("gp", "dve", "act", "gp", "dve")  # gamma engine cycle over groups


def _coeffs(weights):
    """Per-kernel coefficients of out = Cab*a*b + Ca*a + Cb*b + C1."""
    w = [weights[:, i].astype(np.float64) for i in range(16)]
    cab = w[1] - w[2] - w[4] - 2 * w[6] - w[7] + w[8] + 2 * w[9] + w[11] + w[13] - w[14]
    ca = w[2] + w[3] + w[6] + w[7] - w[8] - w[9] - w[12] - w[13]
    cb = w[4] + w[5] + w[6] + w[7] - w[8] - w[9] - w[10] - w[11]
    c1 = w[8] + w[9] + w[10] + w[11] + w[12] + w[13] + w[14] + w[15]
    return cab, ca, cb, c1


def _plan(pairs_a, pairs_b, weights, x0):
    """Host-side schedule.  x0 = x[0] ([C,H,W] fp32) drives the bf16 error
    simulation.  Returns (plans, (runlist, ncols), groups, order, gcol):
      plans[k] = (k, base, a_src, b_src, path, scal, gamma)
      groups   = list of (ks, dt, geng) in emission order; dt in ('bf','f32')
      order    = flattened group k's (device k order); gcol = [H,K] gamma table
    """
    import ml_dtypes

    cab, ca, cb, c1 = _coeffs(weights)
    keys = {}  # (shift, chan) -> use count; shift != 0
    raw = []
    for k in range(K):
        ha, wa, cca = int(pairs_a[k][0]), int(pairs_a[k][1]), int(pairs_a[k][2])
        hb, wb, ccb = int(pairs_b[k][0]), int(pairs_b[k][1]), int(pairs_b[k][2])
        if ha == hb:
            base = ha
            a_key, b_key = (0, cca), (0, ccb)
        else:
            # shifting either side keeps that copy's invalid rows inside the
            # junk-lane range (min_h + |delta| <= 2); reuse existing columns.
            if ha < hb:  # a is the smaller-h side
                neg = ((ha - hb, cca), True, hb)  # (col key, shifts_a, base)
                pos = ((hb - ha, ccb), False, ha)
            else:
                neg = ((hb - ha, ccb), False, ha)
                pos = ((ha - hb, cca), True, hb)
            key, shift_a, base = pos if (pos[0] in keys and neg[0] not in keys) else neg
            keys[key] = keys.get(key, 0) + 1
            if shift_a:
                a_key, b_key = key, (0, ccb)
            else:
                a_key, b_key = (0, cca), key

        kab, kka, kkb, kk1 = float(cab[k]), float(ca[k]), float(cb[k]), float(c1[k])
        if abs(kab) <= 1e-7:
            path, scal, gamma = "linear", (kka, kkb, kk1), 0.0
        elif abs(kkb) <= 50.0 * abs(kab) and abs(kka * kkb) <= 50.0 * abs(kab):
            path = "fact"
            scal = (kab, kka, kkb / kab)
            gamma = kk1 - kka * kkb / kab
        else:
            path, scal, gamma = "exact", (kab, kka, kkb, kk1), 0.0
        raw.append((k, base, a_key, wa, b_key, wb, path, scal, gamma))

    # bf16 routing: simulate the exact device pipeline on batch 0
    xq = x0.astype(ml_dtypes.bfloat16).astype(np.float32)

    def bf(a):
        return a.astype(ml_dtypes.bfloat16).astype(np.float32)

    errs = np.full(K, np.inf)
    absmax = 0.0
    for k in range(K):
        ha, wa, cca = int(pairs_a[k][0]), int(pairs_a[k][1]), int(pairs_a[k][2])
        hb, wb, ccb = int(pairs_b[k][0]), int(pairs_b[k][1]), int(pairs_b[k][2])
        A = x0[cca, ha : ha + OH, wa : wa + OW].astype(np.float64)
        Bv = x0[ccb, hb : hb + OH, wb : wb + OW].astype(np.float64)
        ref = cab[k] * A * Bv + ca[k] * A + cb[k] * Bv + c1[k]
        absmax = max(absmax, np.abs(ref).max())
        if raw[k][6] != "fact":
            continue  # linear/exact stay fp32
        kab, kka, alpha = raw[k][7]
        gamma = raw[k][8]
        Aq = xq[cca, ha : ha + OH, wa : wa + OW]
        Bq = xq[ccb, hb : hb + OH, wb : wb + OW]
        b2 = bf(np.float32(kab) * Bq + np.float32(kka))
        T = bf((Aq + np.float32(alpha)) * b2)
        outq = T.astype(np.float64) + gamma
        errs[k] = np.abs(outq - ref).max()
    dts = ["bf" if errs[k] <= BF16_ERR_THRESH * max(absmax, 1.0) else "f32"
           for k in range(K)]

    # consolidate shifted columns into gap-bridged contiguous c-runs.
    # Prefer FEW runs (SWDGE issue cost ~0.7us each) over few columns.
    def build_runs(gaptol):
        runs, cmap, total = [], {}, 0
        for s in sorted({sc[0] for sc in keys}):
            cs = sorted(c for (s2, c) in keys if s2 == s)
            i = 0
            while i < len(cs):
                j = i
                while j + 1 < len(cs) and cs[j + 1] - cs[j] <= gaptol:
                    j += 1
                c0, cl = cs[i], cs[j]
                for c in range(c0, cl + 1):
                    cmap[(s, c)] = total + (c - c0)
                runs.append((s, c0, cl, total))
                total += cl - c0 + 1
                i = j + 1
        return runs, cmap, total

    for gaptol in (6, 4, 2, 1, 0):
        runlist, cmap, ncols = build_runs(gaptol)
        if ncols <= 100:
            break

    plans = []
    for (k, base, a_key, wa, b_key, wb, path, scal, gamma) in raw:
        a_src = (False, a_key[1], wa) if a_key[0] == 0 else (True, cmap[a_key], wa)
        b_src = (False, b_key[1], wb) if b_key[0] == 0 else (True, cmap[b_key], wb)
        plans.append((k, base, a_src, b_src, path, scal, gamma))

    # order: bf16 kernels first, unshifted before shifted (S lands after X),
    # base-major for batched same-base stores, then by X-chunk need so the
    # first groups only touch early chunks.
    def chunk_need(k):
        need = 0
        for src in (plans[k][2], plans[k][3]):
            if not src[0]:
                need = max(need, src[1] // CSZ)
        return need

    def shifted_any(k):
        return plans[k][2][0] or plans[k][3][0]

    order = sorted(
        range(K),
        key=lambda k: (dts[k] == "f32", shifted_any(k), plans[k][1], chunk_need(k), k),
    )
    # groups of <= GRP, never crossing a dtype boundary
    groups = []
    i = 0
    while i < K:
        dt = dts[order[i]]
        j = i
        while j < K and j - i < GRP and dts[order[j]] == dt:
            j += 1
        geng = GSPLIT[len(groups) % len(GSPLIT)]
        groups.append((order[i:j], dt, geng))
        i = j

    gcol = np.zeros((H, K), np.float32)
    for pos, k in enumerate(order):
        gcol[:, pos] = plans[k][6]
    return plans, (runlist, ncols), groups, order, gcol


def _build(pairs_a, pairs_b, weights, x0):
    import concourse.bacc as bacc
    import concourse.mybir as mybir
    from concourse.tile import TileContext

    f32 = mybir.dt.float32
    bf16 = mybir.dt.bfloat16
    Copy = mybir.ActivationFunctionType.Copy
    add, mult = mybir.AluOpType.add, mybir.AluOpType.mult

    plans, (runlist, ncols), groups, order, gcol_np = _plan(
        pairs_a, pairs_b, weights, x0
    )
    ncols = max(1, ncols)

    if ncols > 110:
        raise RuntimeError(f"shifted-column budget exceeded: {ncols}")

    nc = bacc.Bacc()
    # h-major DRAM layout: per-partition rows of every load are contiguous.
    x = nc.dram_tensor("x", [H, C, BPC, W], bf16, kind="ExternalInput")
    gcd = nc.dram_tensor("gcol", [H, K], f32, kind="ExternalInput")
    # oh-major output: batched group stores write ~8KB contiguous per
    # partition; host transposes back.
    out = nc.dram_tensor("out", [OH, K, BPC, OW], f32, kind="ExternalOutput")

    with TileContext(nc) as tc:
        with (
            tc.tile_pool(name="xp", bufs=1) as xp,
            tc.tile_pool(name="bp", bufs=8) as bp,
            tc.tile_pool(name="tp", bufs=3) as tp,
            tc.tile_pool(name="op", bufs=3) as op,
        ):
            xr = x.rearrange("h c b w -> h c (b w)")
            X = xp.tile([H, C * BPC * W], bf16)
            Xv = X.rearrange("p (c b w) -> p c b w", c=C, b=BPC)
            Xf = X.rearrange("p (c q) -> p c q", c=C)
            for ci in range(NXCHUNK):
                nc.sync.dma_start(
                    out=Xf[:, ci * CSZ : (ci + 1) * CSZ],
                    in_=xr[:, ci * CSZ : (ci + 1) * CSZ],
                )

            S = xp.tile([H, ncols * BPC * W], bf16)
            Sv = S.rearrange("p (j b w) -> p j b w", j=ncols, b=BPC)
            Sf = S.rearrange("p (j q) -> p j q", j=ncols)
            Gc = xp.tile([H, K], f32)
            with tc.high_priority():
                # finite filler for shifted-run head/tail rows (junk lanes)
                for d0 in range(0, ncols, C):
                    n = min(C, ncols - d0)
                    nc.gpsimd.dma_start(out=Sf[0:2, d0 : d0 + n], in_=xr[0:2, 0:n])
                    nc.gpsimd.dma_start(
                        out=Sf[H - 2 : H, d0 : d0 + n], in_=xr[0:2, 0:n]
                    )
                for (s, c0, cl, d0) in runlist:
                    # S[p, d0+i] = x[c0+i, p+s]
                    n = cl - c0 + 1
                    if s < 0:
                        nc.gpsimd.dma_start(
                            out=Sf[-s:H, d0 : d0 + n], in_=xr[0 : H + s, c0 : c0 + n]
                        )
                    else:
                        nc.gpsimd.dma_start(
                            out=Sf[0 : H - s, d0 : d0 + n], in_=xr[s:H, c0 : c0 + n]
                        )
                nc.gpsimd.dma_start(out=Gc, in_=gcd[:, :])

            out_kb = out.rearrange("oh k b ow -> oh (k b) ow")
            fd = BPC * OW

            def odd(ap):
                # [p, (4*63)] view: odd innermost dim caps DVE TS at 2x_1P so
                # it never grabs the DVE/GpSimd shared SBUF port pair.
                return ap.rearrange("p (a q) -> p a q", a=4)

            def emit_gamma_and_store(pos0, ks, geng, T, O):
                # deferred one group so cross-engine waits are pre-satisfied
                for j, k in enumerate(ks):
                    _, base, _, _, path, scal, gamma = plans[k]
                    cnt = base + OH
                    slot = T[0:cnt, j * fd : (j + 1) * fd]
                    osl = O[0:cnt, j * fd : (j + 1) * fd]
                    if geng == "act":
                        nc.scalar.activation(osl, slot, Copy, bias=gamma, scale=1.0)
                    elif geng == "dve":
                        nc.vector.tensor_scalar(odd(osl), odd(slot), gamma, None, add)
                    else:
                        gb = Gc[0:cnt, pos0 + j : pos0 + j + 1].broadcast_to([cnt, fd])
                        nc.gpsimd.tensor_tensor(osl, slot, gb, add)
                # batched stores per same-base run: SWDGE on the GpSimd queue
                i = 0
                while i < len(ks):
                    base = plans[ks[i]][1]
                    i2 = i
                    while i2 < len(ks) and plans[ks[i2]][1] == base:
                        i2 += 1
                    src = O[base : base + OH, i * fd : i2 * fd].rearrange(
                        "p (kb w) -> p kb w", w=OW
                    )
                    dst = out_kb[:, (pos0 + i) * BPC : (pos0 + i2) * BPC]
                    nc.gpsimd.dma_start(out=dst, in_=src)
                    i = i2

            pending = None
            pos0 = 0
            for gi, (ks, gdt, geng) in enumerate(groups):
                dt = bf16 if gdt == "bf" else f32
                T = tp.tile([H, GRP * fd], dt, tag="t", name=f"t_{gi}")
                O = op.tile([H, GRP * fd], f32, tag="o", name=f"o_{gi}")

                for j, k in enumerate(ks):
                    _, base, a_src, b_src, path, scal, gamma = plans[k]
                    cnt = base + OH

                    def view(src):
                        shifted, idx, woff = src
                        t = Sv if shifted else Xv
                        return t[0:cnt, idx, :, woff : woff + OW]

                    Av, Bv = view(a_src), view(b_src)
                    slot = T[0:cnt, j * fd : (j + 1) * fd]
                    slotv = slot.rearrange("p (b w) -> p b w", b=BPC)
                    b2 = bp.tile([H, fd], dt, tag="b2", name=f"b2_{k}")
                    b2v = b2.rearrange("p (b w) -> p b w", b=BPC)[0:cnt]

                    if path == "fact":
                        kab, kka, alpha = scal
                        if geng == "act":
                            # offload this group's affine to DVE: its gamma
                            # pass runs on ScalarE.  Odd-inner APs (63) keep
                            # TS off the 2-port modes.
                            bo = b2v.rearrange("p b (c q) -> p b c q", q=63)
                            Bo = Bv.rearrange("p b (c q) -> p b c q", q=63)
                            nc.vector.tensor_scalar(bo, Bo, kab, kka, mult, add)
                        else:
                            nc.scalar.activation(b2v, Bv, Copy, bias=kka, scale=kab)
                        nc.vector.scalar_tensor_tensor(slotv, Av, alpha, b2v, add, mult)
                    else:  # linear/exact: slot = Ca*A + (Cb*B + C1)
                        if path == "linear":
                            kka, kkb, kk1 = scal
                        else:
                            kab, kka, kkb, kk1 = scal
                        nc.scalar.activation(b2v, Bv, Copy, bias=kk1, scale=kkb)
                        nc.vector.scalar_tensor_tensor(slotv, Av, kka, b2v, mult, add)
                        if path == "exact":  # += (Cab*B)*A
                            p2 = bp.tile([H, fd], dt, tag="b2", name=f"p2_{k}")
                            p2v = p2.rearrange("p (b w) -> p b w", b=BPC)[0:cnt]
                            nc.vector.scalar_tensor_tensor(p2v, Bv, kab, Av, mult, mult)
                            nc.vector.tensor_tensor(slot, slot, p2[0:cnt], add)

                if pending is not None:
                    emit_gamma_and_store(*pending)
                pending = (pos0, ks, geng, T, O)
                pos0 += len(ks)
            if pending is not None:
                emit_gamma_and_store(*pending)
    nc.compile()
    return nc


def kernel(x, pairs_a, pairs_b, weights):
    import ml_dtypes
    from concourse.bass_utils import run_bass_kernel_spmd

    x = np.ascontiguousarray(np.asarray(x), dtype=np.float32)
    pa = np.asarray(pairs_a).astype(np.int64)
    pb = np.asarray(pairs_b).astype(np.int64)
    w = np.asarray(weights).astype(np.float32)

    nc = _build(pa, pb, w, x[0])
    _, _, _, order, gcol = _plan(pa, pb, w, x[0])
    xq = x.astype(ml_dtypes.bfloat16)
    in_maps = [
        {
            "x": np.ascontiguousarray(
                xq[i * BPC : (i + 1) * BPC].transpose(2, 1, 0, 3)
            ),
            "gcol": gcol,
        }
        for i in range(NCORES)
    ]
    res = run_bass_kernel_spmd(nc, in_maps, core_ids=list(range(NCORES)))
    # device layout [OH, K(sorted), BPC, OW] per core -> [B, K, OH, OW]
    full = np.concatenate([r["out"] for r in res.results], axis=2)  # [OH,K,B,OW]
    pos = np.empty(K, np.int64)
    pos[np.asarray(order)] = np.arange(K)
    return np.ascontiguousarray(full[:, pos].transpose(2, 1, 0, 3))


# revision 14
# speedup vs baseline: 1.4349x; 1.4349x over previous
"""Trainium2 Bass kernel for nn_LogicConvSparseMatrix.

Math: the reference's 15-term weighted logic-op sum collapses to

    out[b,k] = C_ab[k]*A*B + C_a[k]*A + C_b[k]*B + C_1[k]

where A = x[b, ca_k, ha_k+oh, wa_k+ow], B = x[b, cb_k, hb_k+oh, wb_k+ow]
are shifted 126x126 windows.  With alpha = C_b/C_ab, gamma = C_1 -
C_a*C_b/C_ab this factors into

    out = (A + alpha) * (C_ab*B + C_a) + gamma

Per kernel k, three element passes:
  1. affine:  B2 = C_ab*B + C_a           (ScalarE activation or DVE TS)
  2. product: T = (A + alpha) * B2        (DVE scalar_tensor_tensor)
  3. +gamma -> fp32 output staging tile O (ScalarE / DVE TS / GpSimd TT,
     round-robin per group for load balance)

Index pairs are known at build time, so gathers are compile-time SBUF
views of X[p=h, (c,b,w)].  Compute-engine SBUF operands may only start at
partition 0/32/64/96; the relative h-shift between the two windows is
materialized as shifted column copies loaded straight from DRAM.

dtype: x is pre-cast to bf16 on host (halves load bytes; DVE gets 2x
mode).  Intermediates (b2, T) are bf16 only for kernels whose simulated
bf16 error (on batch 0) stays under BF16_ERR_THRESH of the output
absmax; risky kernels (large alpha/gamma amplification) run with fp32
intermediates.  The gamma pass always writes the fp32 staging tile O.

DMA routing (all measured on HW):
  - X loads: 8 c-chunks on the SP HWDGE ring ([H,C,BPC,W] h-major DRAM
    layout -> 8KB-per-partition contiguous descriptors, line rate).
    Compute on early channels starts while later chunks stream.
  - shifted-run/filler/gcol loads: SWDGE (gpsimd queue) under
    tc.high_priority().  Their small 256KB-strided descriptors all hash
    onto a single SDMA engine under HWDGE (measured 23 GB/s); SWDGE
    round-robins descriptors across all 16 engines.  high_priority stops
    the Tile scheduler from parking them behind early compute.
  - stores: SWDGE from the GpSimd queue (issue ~0.7us, transfer async).
    Device output layout [OH, Ksorted, BPC, OW] makes one batched
    same-base group store a run of ~8KB contiguous per-partition
    descriptors (bursts at line rate).  The host inverse-permutes and
    transposes back to [B, K, OH, OW].

No DVE op may use a 2-port perf mode (fp32 copy / even-innermost bf16
tensor_scalar): that locks GpSimd out of SBUF and starves SWDGE
descriptor generation.  All DVE tensor_scalar APs are reshaped to an odd
innermost dim (63) to cap them at 2x_1P.

Sharding: data-parallel over batch, 2 batch items per core, 8 cores.
"""

import numpy as np

B, C, H, W = 16, 64, 128, 128
K = 128
RH = RW = 3
OH, OW = H - RH + 1, W - RW + 1
NCORES = 8
BPC = B // NCORES

GRP = 8  # kernels per store group
NXCHUNK = 8  # X load chunks on the SP ring
CSZ = C // NXCHUNK
BF16_ERR_THRESH = 1.5e-3  # sim bf16 err (rel to absmax) above which k goes fp32


def _geng(gi):
    # gamma-engine per group: measured costs gamma-dve 250ns, gamma-act
    # 480ns, gamma-gp 690ns per k -> gp/dve alternate, two act groups.
    if gi in (2, 9):
        return "act"
    return "gp" if gi % 2 == 0 else "dve"


def _coeffs(weights):
    """Per-kernel coefficients of out = Cab*a*b + Ca*a + Cb*b + C1."""
    w = [weights[:, i].astype(np.float64) for i in range(16)]
    cab = w[1] - w[2] - w[4] - 2 * w[6] - w[7] + w[8] + 2 * w[9] + w[11] + w[13] - w[14]
    ca = w[2] + w[3] + w[6] + w[7] - w[8] - w[9] - w[12] - w[13]
    cb = w[4] + w[5] + w[6] + w[7] - w[8] - w[9] - w[10] - w[11]
    c1 = w[8] + w[9] + w[10] + w[11] + w[12] + w[13] + w[14] + w[15]
    return cab, ca, cb, c1


def _plan(pairs_a, pairs_b, weights, x0):
    """Host-side schedule.  x0 = x[0] ([C,H,W] fp32) drives the bf16 error
    simulation.  Returns (plans, (runlist, ncols), groups, order, gcol):
      plans[k] = (k, base, a_src, b_src, path, scal, gamma)
      groups   = list of (ks, dt, geng) in emission order; dt in ('bf','f32')
      order    = flattened group k's (device k order); gcol = [H,K] gamma table
    """
    import ml_dtypes

    cab, ca, cb, c1 = _coeffs(weights)
    keys = {}  # (shift, chan) -> use count; shift != 0
    raw = []
    for k in range(K):
        ha, wa, cca = int(pairs_a[k][0]), int(pairs_a[k][1]), int(pairs_a[k][2])
        hb, wb, ccb = int(pairs_b[k][0]), int(pairs_b[k][1]), int(pairs_b[k][2])
        if ha == hb:
            base = ha
            a_key, b_key = (0, cca), (0, ccb)
        else:
            # shifting either side keeps that copy's invalid rows inside the
            # junk-lane range (min_h + |delta| <= 2); reuse existing columns.
            if ha < hb:  # a is the smaller-h side
                neg = ((ha - hb, cca), True, hb)  # (col key, shifts_a, base)
                pos = ((hb - ha, ccb), False, ha)
            else:
                neg = ((hb - ha, ccb), False, ha)
                pos = ((ha - hb, cca), True, hb)
            key, shift_a, base = pos if (pos[0] in keys and neg[0] not in keys) else neg
            keys[key] = keys.get(key, 0) + 1
            if shift_a:
                a_key, b_key = key, (0, ccb)
            else:
                a_key, b_key = (0, cca), key

        kab, kka, kkb, kk1 = float(cab[k]), float(ca[k]), float(cb[k]), float(c1[k])
        if abs(kab) <= 1e-7:
            path, scal, gamma = "linear", (kka, kkb, kk1), 0.0
        elif abs(kkb) <= 50.0 * abs(kab) and abs(kka * kkb) <= 50.0 * abs(kab):
            path = "fact"
            scal = (kab, kka, kkb / kab)
            gamma = kk1 - kka * kkb / kab
        else:
            path, scal, gamma = "exact", (kab, kka, kkb, kk1), 0.0
        raw.append((k, base, a_key, wa, b_key, wb, path, scal, gamma))

    # bf16 routing: simulate the exact device pipeline on batch 0
    xq = x0.astype(ml_dtypes.bfloat16).astype(np.float32)

    def bf(a):
        return a.astype(ml_dtypes.bfloat16).astype(np.float32)

    errs = np.full(K, np.inf)
    absmax = 0.0
    for k in range(K):
        ha, wa, cca = int(pairs_a[k][0]), int(pairs_a[k][1]), int(pairs_a[k][2])
        hb, wb, ccb = int(pairs_b[k][0]), int(pairs_b[k][1]), int(pairs_b[k][2])
        A = x0[cca, ha : ha + OH, wa : wa + OW].astype(np.float64)
        Bv = x0[ccb, hb : hb + OH, wb : wb + OW].astype(np.float64)
        ref = cab[k] * A * Bv + ca[k] * A + cb[k] * Bv + c1[k]
        absmax = max(absmax, np.abs(ref).max())
        if raw[k][6] != "fact":
            continue  # linear/exact stay fp32
        kab, kka, alpha = raw[k][7]
        gamma = raw[k][8]
        Aq = xq[cca, ha : ha + OH, wa : wa + OW]
        Bq = xq[ccb, hb : hb + OH, wb : wb + OW]
        b2 = bf(np.float32(kab) * Bq + np.float32(kka))
        T = bf((Aq + np.float32(alpha)) * b2)
        outq = T.astype(np.float64) + gamma
        errs[k] = np.abs(outq - ref).max()
    dts = ["bf" if errs[k] <= BF16_ERR_THRESH * max(absmax, 1.0) else "f32"
           for k in range(K)]

    # consolidate shifted columns into gap-bridged contiguous c-runs.
    # Prefer FEW runs (SWDGE issue cost ~0.7us each) over few columns.
    def build_runs(gaptol):
        runs, cmap, total = [], {}, 0
        for s in sorted({sc[0] for sc in keys}):
            cs = sorted(c for (s2, c) in keys if s2 == s)
            i = 0
            while i < len(cs):
                j = i
                while j + 1 < len(cs) and cs[j + 1] - cs[j] <= gaptol:
                    j += 1
                c0, cl = cs[i], cs[j]
                for c in range(c0, cl + 1):
                    cmap[(s, c)] = total + (c - c0)
                runs.append((s, c0, cl, total))
                total += cl - c0 + 1
                i = j + 1
        return runs, cmap, total

    for gaptol in (6, 4, 2, 1, 0):
        runlist, cmap, ncols = build_runs(gaptol)
        if ncols <= 100:
            break

    plans = []
    for (k, base, a_key, wa, b_key, wb, path, scal, gamma) in raw:
        a_src = (False, a_key[1], wa) if a_key[0] == 0 else (True, cmap[a_key], wa)
        b_src = (False, b_key[1], wb) if b_key[0] == 0 else (True, cmap[b_key], wb)
        plans.append((k, base, a_src, b_src, path, scal, gamma))

    # order: bf16 kernels first, unshifted before shifted (S lands after X),
    # base-major for batched same-base stores, then by X-chunk need so the
    # first groups only touch early chunks.
    def chunk_need(k):
        need = 0
        for src in (plans[k][2], plans[k][3]):
            if not src[0]:
                need = max(need, src[1] // CSZ)
        return need

    def shifted_any(k):
        return plans[k][2][0] or plans[k][3][0]

    order = sorted(
        range(K),
        key=lambda k: (dts[k] == "f32", shifted_any(k), plans[k][1], chunk_need(k), k),
    )
    # groups of <= GRP, never crossing a dtype boundary
    groups = []
    i = 0
    while i < K:
        dt = dts[order[i]]
        j = i
        while j < K and j - i < GRP and dts[order[j]] == dt:
            j += 1
        groups.append((order[i:j], dt, _geng(len(groups))))
        i = j

    gcol = np.zeros((H, K), np.float32)
    for pos, k in enumerate(order):
        gcol[:, pos] = plans[k][6]
    return plans, (runlist, ncols), groups, order, gcol


def _build(pairs_a, pairs_b, weights, x0):
    import concourse.bacc as bacc
    import concourse.mybir as mybir
    from concourse.tile import TileContext

    f32 = mybir.dt.float32
    bf16 = mybir.dt.bfloat16
    Copy = mybir.ActivationFunctionType.Copy
    add, mult = mybir.AluOpType.add, mybir.AluOpType.mult

    plans, (runlist, ncols), groups, order, gcol_np = _plan(
        pairs_a, pairs_b, weights, x0
    )
    ncols = max(1, ncols)

    if ncols > 110:
        raise RuntimeError(f"shifted-column budget exceeded: {ncols}")

    nc = bacc.Bacc()
    # h-major DRAM layout: per-partition rows of every load are contiguous.
    x = nc.dram_tensor("x", [H, C, BPC, W], bf16, kind="ExternalInput")
    gcd = nc.dram_tensor("gcol", [H, K], f32, kind="ExternalInput")
    # oh-major output: batched group stores write ~8KB contiguous per
    # partition; host transposes back.
    out = nc.dram_tensor("out", [OH, K, BPC, OW], f32, kind="ExternalOutput")

    with TileContext(nc) as tc:
        with (
            tc.tile_pool(name="xp", bufs=1) as xp,
            tc.tile_pool(name="bp", bufs=8) as bp,
            tc.tile_pool(name="tp", bufs=3) as tp,
            tc.tile_pool(name="op", bufs=3) as op,
        ):
            xr = x.rearrange("h c b w -> h c (b w)")
            X = xp.tile([H, C * BPC * W], bf16)
            Xv = X.rearrange("p (c b w) -> p c b w", c=C, b=BPC)
            Xf = X.rearrange("p (c q) -> p c q", c=C)
            for ci in range(NXCHUNK):
                nc.sync.dma_start(
                    out=Xf[:, ci * CSZ : (ci + 1) * CSZ],
                    in_=xr[:, ci * CSZ : (ci + 1) * CSZ],
                )

            S = xp.tile([H, ncols * BPC * W], bf16)
            Sv = S.rearrange("p (j b w) -> p j b w", j=ncols, b=BPC)
            Sf = S.rearrange("p (j q) -> p j q", j=ncols)
            Gc = xp.tile([H, K], f32)
            with tc.high_priority(offset=1_000_000):
                # finite filler for exactly the junk rows of each run's
                # columns (head rows [0:-s] for s<0, tail [H-s:H] for s>0).
                # Never touches rows the run writes, so filler/run order is
                # free (high_priority flattens priorities; an overlapping
                # filler racing a run corrupted valid rows).
                for (s, c0, cl, d0) in runlist:
                    n = cl - c0 + 1
                    if s < 0:
                        nc.gpsimd.dma_start(
                            out=Sf[0:-s, d0 : d0 + n], in_=xr[0:-s, 0:n]
                        )
                    else:
                        nc.gpsimd.dma_start(
                            out=Sf[H - s : H, d0 : d0 + n], in_=xr[0:s, 0:n]
                        )
                for (s, c0, cl, d0) in runlist:
                    # S[p, d0+i] = x[c0+i, p+s].  Chunked to <=8 columns:
                    # per-partition descriptors >4KB sometimes hash onto a
                    # single SDMA engine (measured: a 22KB-descriptor run
                    # serialized at 27 GB/s for 106us); <=4KB always spreads.
                    for cc in range(c0, cl + 1, 8):
                        ce = min(cc + 8, cl + 1)
                        n = ce - cc
                        dd = d0 + (cc - c0)
                        if s < 0:
                            nc.gpsimd.dma_start(
                                out=Sf[-s:H, dd : dd + n],
                                in_=xr[0 : H + s, cc:ce],
                            )
                        else:
                            nc.gpsimd.dma_start(
                                out=Sf[0 : H - s, dd : dd + n],
                                in_=xr[s:H, cc:ce],
                            )
                nc.gpsimd.dma_start(out=Gc, in_=gcd[:, :])

            out_kb = out.rearrange("oh k b ow -> oh (k b) ow")
            fd = BPC * OW

            def odd(ap):
                # [p, (4*63)] view: odd innermost dim caps DVE TS at 2x_1P so
                # it never grabs the DVE/GpSimd shared SBUF port pair.
                return ap.rearrange("p (a q) -> p a q", a=4)

            def emit_gamma_and_store(pos0, ks, geng, T, O):
                # deferred one group so cross-engine waits are pre-satisfied
                for j, k in enumerate(ks):
                    _, base, _, _, path, scal, gamma = plans[k]
                    cnt = base + OH
                    slot = T[0:cnt, j * fd : (j + 1) * fd]
                    osl = O[0:cnt, j * fd : (j + 1) * fd]
                    if geng == "act":
                        nc.scalar.activation(osl, slot, Copy, bias=gamma, scale=1.0)
                    elif geng == "dve":
                        nc.vector.tensor_scalar(odd(osl), odd(slot), gamma, None, add)
                    else:
                        gb = Gc[0:cnt, pos0 + j : pos0 + j + 1].broadcast_to([cnt, fd])
                        nc.gpsimd.tensor_tensor(osl, slot, gb, add)
                # batched stores per same-base run: SWDGE on the GpSimd queue
                i = 0
                while i < len(ks):
                    base = plans[ks[i]][1]
                    i2 = i
                    while i2 < len(ks) and plans[ks[i2]][1] == base:
                        i2 += 1
                    src = O[base : base + OH, i * fd : i2 * fd].rearrange(
                        "p (kb w) -> p kb w", w=OW
                    )
                    dst = out_kb[:, (pos0 + i) * BPC : (pos0 + i2) * BPC]
                    nc.gpsimd.dma_start(out=dst, in_=src)
                    i = i2

            pending = None
            pos0 = 0
            for gi, (ks, gdt, geng) in enumerate(groups):
                dt = bf16 if gdt == "bf" else f32
                T = tp.tile([H, GRP * fd], dt, tag="t", name=f"t_{gi}")
                O = op.tile([H, GRP * fd], f32, tag="o", name=f"o_{gi}")

                for j, k in enumerate(ks):
                    _, base, a_src, b_src, path, scal, gamma = plans[k]
                    cnt = base + OH

                    def view(src):
                        shifted, idx, woff = src
                        t = Sv if shifted else Xv
                        return t[0:cnt, idx, :, woff : woff + OW]

                    Av, Bv = view(a_src), view(b_src)
                    slot = T[0:cnt, j * fd : (j + 1) * fd]
                    slotv = slot.rearrange("p (b w) -> p b w", b=BPC)
                    b2 = bp.tile([H, fd], dt, tag="b2", name=f"b2_{k}")
                    b2v = b2.rearrange("p (b w) -> p b w", b=BPC)[0:cnt]

                    if path == "fact":
                        kab, kka, alpha = scal
                        nc.scalar.activation(b2v, Bv, Copy, bias=kka, scale=kab)
                        nc.vector.scalar_tensor_tensor(slotv, Av, alpha, b2v, add, mult)
                    else:  # linear/exact: slot = Ca*A + (Cb*B + C1)
                        if path == "linear":
                            kka, kkb, kk1 = scal
                        else:
                            kab, kka, kkb, kk1 = scal
                        nc.scalar.activation(b2v, Bv, Copy, bias=kk1, scale=kkb)
                        nc.vector.scalar_tensor_tensor(slotv, Av, kka, b2v, mult, add)
                        if path == "exact":  # += (Cab*B)*A
                            p2 = bp.tile([H, fd], dt, tag="b2", name=f"p2_{k}")
                            p2v = p2.rearrange("p (b w) -> p b w", b=BPC)[0:cnt]
                            nc.vector.scalar_tensor_tensor(p2v, Bv, kab, Av, mult, mult)
                            nc.vector.tensor_tensor(slot, slot, p2[0:cnt], add)

                if pending is not None:
                    emit_gamma_and_store(*pending)
                pending = (pos0, ks, geng, T, O)
                pos0 += len(ks)
            if pending is not None:
                emit_gamma_and_store(*pending)
    nc.compile()
    return nc


def kernel(x, pairs_a, pairs_b, weights):
    import ml_dtypes
    from concourse.bass_utils import run_bass_kernel_spmd

    x = np.ascontiguousarray(np.asarray(x), dtype=np.float32)
    pa = np.asarray(pairs_a).astype(np.int64)
    pb = np.asarray(pairs_b).astype(np.int64)
    w = np.asarray(weights).astype(np.float32)

    nc = _build(pa, pb, w, x[0])
    _, _, _, order, gcol = _plan(pa, pb, w, x[0])
    xq = x.astype(ml_dtypes.bfloat16)
    in_maps = [
        {
            "x": np.ascontiguousarray(
                xq[i * BPC : (i + 1) * BPC].transpose(2, 1, 0, 3)
            ),
            "gcol": gcol,
        }
        for i in range(NCORES)
    ]
    res = run_bass_kernel_spmd(nc, in_maps, core_ids=list(range(NCORES)))
    # device layout [OH, K(sorted), BPC, OW] per core -> [B, K, OH, OW]
    full = np.concatenate([r["out"] for r in res.results], axis=2)  # [OH,K,B,OW]
    pos = np.empty(K, np.int64)
    pos[np.asarray(order)] = np.arange(K)
    return np.ascontiguousarray(full[:, pos].transpose(2, 1, 0, 3))


# revision 18
# speedup vs baseline: 1.7784x; 1.2394x over previous
"""Trainium2 Bass kernel for nn_LogicConvSparseMatrix.

Math: the reference's 15-term weighted logic-op sum collapses to

    out[b,k] = C_ab[k]*A*B + C_a[k]*A + C_b[k]*B + C_1[k]

where A = x[b, ca_k, ha_k+oh, wa_k+ow], B = x[b, cb_k, hb_k+oh, wb_k+ow]
are shifted 126x126 windows.  With alpha = C_b/C_ab, gamma = C_1 -
C_a*C_b/C_ab this factors into

    out = (A + alpha) * (C_ab*B + C_a) + gamma

Per kernel k, three element passes:
  1. affine:  B2 = C_ab*B + C_a           (ScalarE activation or DVE TS)
  2. product: T = (A + alpha) * B2        (DVE scalar_tensor_tensor)
  3. +gamma -> fp32 output staging tile O (ScalarE / DVE TS / GpSimd TT,
     round-robin per group for load balance)

Index pairs are known at build time, so gathers are compile-time SBUF
views of X[p=h, (c,b,w)].  Compute-engine SBUF operands may only start at
partition 0/32/64/96; the relative h-shift between the two windows is
materialized as shifted column copies loaded straight from DRAM.

dtype: x is pre-cast to bf16 on host (halves load bytes; DVE gets 2x
mode).  Intermediates (b2, T) are bf16 only for kernels whose simulated
bf16 error (on batch 0) stays under BF16_ERR_THRESH of the output
absmax; risky kernels (large alpha/gamma amplification) run with fp32
intermediates.  The gamma pass always writes the fp32 staging tile O.

DMA routing (all measured on HW):
  - X loads: 8 c-chunks on the SP HWDGE ring ([H,C,BPC,W] h-major DRAM
    layout -> 8KB-per-partition contiguous descriptors, line rate).
    Compute on early channels starts while later chunks stream.
  - shifted-run/filler/gcol loads: SWDGE (gpsimd queue) under
    tc.high_priority().  Their small 256KB-strided descriptors all hash
    onto a single SDMA engine under HWDGE (measured 23 GB/s); SWDGE
    round-robins descriptors across all 16 engines.  high_priority stops
    the Tile scheduler from parking them behind early compute.
  - stores: SWDGE from the GpSimd queue (issue ~0.7us, transfer async).
    Device output layout [OH, Ksorted, BPC, OW] makes one batched
    same-base group store a run of ~8KB contiguous per-partition
    descriptors (bursts at line rate).  The host inverse-permutes and
    transposes back to [B, K, OH, OW].

No DVE op may use a 2-port perf mode (fp32 copy / even-innermost bf16
tensor_scalar): that locks GpSimd out of SBUF and starves SWDGE
descriptor generation.  All DVE tensor_scalar APs are reshaped to an odd
innermost dim (63) to cap them at 2x_1P.

Sharding: data-parallel over batch, 2 batch items per core, 8 cores.
"""

import numpy as np

B, C, H, W = 16, 64, 128, 128
K = 128
RH = RW = 3
OH, OW = H - RH + 1, W - RW + 1
NCORES = 8
BPC = B // NCORES

GRP = 8  # kernels per store group
NXCHUNK = 8  # X load chunks on the SP ring
CSZ = C // NXCHUNK
BF16_ERR_THRESH = 1.5e-3  # sim bf16 err (rel to absmax) above which k goes fp32


def _geng(gi):
    # gamma-engine per group.  No "gp": GpSimd stays a pure DMA engine --
    # running TENSOR_TENSOR there forces a one-time ucode lib swap that
    # must drain all outstanding SWDGE DMAs (measured: first gp gamma
    # delayed to 72us behind store-completion chains).  Measured costs:
    # gamma-dve 250ns, gamma-act 490ns -> 1 act group per 4.
    return "act" if gi % 4 == 3 else "dve"


def _coeffs(weights):
    """Per-kernel coefficients of out = Cab*a*b + Ca*a + Cb*b + C1."""
    w = [weights[:, i].astype(np.float64) for i in range(16)]
    cab = w[1] - w[2] - w[4] - 2 * w[6] - w[7] + w[8] + 2 * w[9] + w[11] + w[13] - w[14]
    ca = w[2] + w[3] + w[6] + w[7] - w[8] - w[9] - w[12] - w[13]
    cb = w[4] + w[5] + w[6] + w[7] - w[8] - w[9] - w[10] - w[11]
    c1 = w[8] + w[9] + w[10] + w[11] + w[12] + w[13] + w[14] + w[15]
    return cab, ca, cb, c1


def _plan(pairs_a, pairs_b, weights, x0):
    """Host-side schedule.  x0 = x[0] ([C,H,W] fp32) drives the bf16 error
    simulation.  Returns (plans, (runlist, ncols), groups, order, gcol):
      plans[k] = (k, base, a_src, b_src, path, scal, gamma)
      groups   = list of (ks, dt, geng) in emission order; dt in ('bf','f32')
      order    = flattened group k's (device k order); gcol = [H,K] gamma table
    """
    import ml_dtypes

    cab, ca, cb, c1 = _coeffs(weights)
    keys = {}  # (shift, chan) -> use count; shift != 0
    raw = []
    for k in range(K):
        ha, wa, cca = int(pairs_a[k][0]), int(pairs_a[k][1]), int(pairs_a[k][2])
        hb, wb, ccb = int(pairs_b[k][0]), int(pairs_b[k][1]), int(pairs_b[k][2])
        if ha == hb:
            base = ha
            a_key, b_key = (0, cca), (0, ccb)
        else:
            # shifting either side keeps that copy's invalid rows inside the
            # junk-lane range (min_h + |delta| <= 2); reuse existing columns.
            if ha < hb:  # a is the smaller-h side
                neg = ((ha - hb, cca), True, hb)  # (col key, shifts_a, base)
                pos = ((hb - ha, ccb), False, ha)
            else:
                neg = ((hb - ha, ccb), False, ha)
                pos = ((ha - hb, cca), True, hb)
            key, shift_a, base = pos if (pos[0] in keys and neg[0] not in keys) else neg
            keys[key] = keys.get(key, 0) + 1
            if shift_a:
                a_key, b_key = key, (0, ccb)
            else:
                a_key, b_key = (0, cca), key

        kab, kka, kkb, kk1 = float(cab[k]), float(ca[k]), float(cb[k]), float(c1[k])
        if abs(kab) <= 1e-7:
            path, scal, gamma = "linear", (kka, kkb, kk1), 0.0
        elif abs(kkb) <= 50.0 * abs(kab) and abs(kka * kkb) <= 50.0 * abs(kab):
            path = "fact"
            scal = (kab, kka, kkb / kab)
            gamma = kk1 - kka * kkb / kab
        else:
            path, scal, gamma = "exact", (kab, kka, kkb, kk1), 0.0
        raw.append((k, base, a_key, wa, b_key, wb, path, scal, gamma))

    # bf16 routing: simulate the exact device pipeline on batch 0
    xq = x0.astype(ml_dtypes.bfloat16).astype(np.float32)

    def bf(a):
        return a.astype(ml_dtypes.bfloat16).astype(np.float32)

    errs = np.full(K, np.inf)
    absmax = 0.0
    for k in range(K):
        ha, wa, cca = int(pairs_a[k][0]), int(pairs_a[k][1]), int(pairs_a[k][2])
        hb, wb, ccb = int(pairs_b[k][0]), int(pairs_b[k][1]), int(pairs_b[k][2])
        A = x0[cca, ha : ha + OH, wa : wa + OW].astype(np.float64)
        Bv = x0[ccb, hb : hb + OH, wb : wb + OW].astype(np.float64)
        ref = cab[k] * A * Bv + ca[k] * A + cb[k] * Bv + c1[k]
        absmax = max(absmax, np.abs(ref).max())
        if raw[k][6] != "fact":
            continue  # linear/exact stay fp32
        kab, kka, alpha = raw[k][7]
        gamma = raw[k][8]
        Aq = xq[cca, ha : ha + OH, wa : wa + OW]
        Bq = xq[ccb, hb : hb + OH, wb : wb + OW]
        b2 = bf(np.float32(kab) * Bq + np.float32(kka))
        T = bf((Aq + np.float32(alpha)) * b2)
        outq = T.astype(np.float64) + gamma
        errs[k] = np.abs(outq - ref).max()
    dts = ["bf" if errs[k] <= BF16_ERR_THRESH * max(absmax, 1.0) else "f32"
           for k in range(K)]

    # consolidate shifted columns into gap-bridged contiguous c-runs.
    # Prefer FEW runs (SWDGE issue cost ~0.7us each) over few columns.
    def build_runs(gaptol):
        runs, cmap, total = [], {}, 0
        for s in sorted({sc[0] for sc in keys}):
            cs = sorted(c for (s2, c) in keys if s2 == s)
            i = 0
            while i < len(cs):
                j = i
                while j + 1 < len(cs) and cs[j + 1] - cs[j] <= gaptol:
                    j += 1
                c0, cl = cs[i], cs[j]
                for c in range(c0, cl + 1):
                    cmap[(s, c)] = total + (c - c0)
                runs.append((s, c0, cl, total))
                total += cl - c0 + 1
                i = j + 1
        return runs, cmap, total

    for gaptol in (6, 4, 2, 1, 0):
        runlist, cmap, ncols = build_runs(gaptol)
        if ncols <= 100:
            break

    plans = []
    for (k, base, a_key, wa, b_key, wb, path, scal, gamma) in raw:
        a_src = (False, a_key[1], wa) if a_key[0] == 0 else (True, cmap[a_key], wa)
        b_src = (False, b_key[1], wb) if b_key[0] == 0 else (True, cmap[b_key], wb)
        plans.append((k, base, a_src, b_src, path, scal, gamma))

    # order: bf16 kernels first, unshifted before shifted (S lands after X),
    # base-major for batched same-base stores, then by X-chunk need so the
    # first groups only touch early chunks.
    def chunk_need(k):
        need = 0
        for src in (plans[k][2], plans[k][3]):
            if not src[0]:
                need = max(need, src[1] // CSZ)
        return need

    def shifted_any(k):
        return plans[k][2][0] or plans[k][3][0]

    order = sorted(
        range(K),
        key=lambda k: (dts[k] == "f32", shifted_any(k), plans[k][1], chunk_need(k), k),
    )
    # groups of <= GRP, never crossing a dtype boundary
    groups = []
    i = 0
    while i < K:
        dt = dts[order[i]]
        j = i
        while j < K and j - i < GRP and dts[order[j]] == dt:
            j += 1
        groups.append((order[i:j], dt, _geng(len(groups))))
        i = j

    gcol = np.zeros((H, K), np.float32)
    for pos, k in enumerate(order):
        gcol[:, pos] = plans[k][6]
    return plans, (runlist, ncols), groups, order, gcol


def _build(pairs_a, pairs_b, weights, x0):
    import concourse.bacc as bacc
    import concourse.mybir as mybir
    from concourse.tile import TileContext

    f32 = mybir.dt.float32
    bf16 = mybir.dt.bfloat16
    Copy = mybir.ActivationFunctionType.Copy
    add, mult = mybir.AluOpType.add, mybir.AluOpType.mult

    plans, (runlist, ncols), groups, order, gcol_np = _plan(
        pairs_a, pairs_b, weights, x0
    )
    ncols = max(1, ncols)

    if ncols > 110:
        raise RuntimeError(f"shifted-column budget exceeded: {ncols}")

    nc = bacc.Bacc()
    # h-major DRAM layout: per-partition rows of every load are contiguous.
    x = nc.dram_tensor("x", [H, C, BPC, W], bf16, kind="ExternalInput")
    gcd = nc.dram_tensor("gcol", [H, K], f32, kind="ExternalInput")
    # oh-major output: batched group stores write ~8KB contiguous per
    # partition; host transposes back.
    out = nc.dram_tensor("out", [OH, K, BPC, OW], f32, kind="ExternalOutput")

    with TileContext(nc) as tc:
        with (
            tc.tile_pool(name="xp", bufs=1) as xp,
            tc.tile_pool(name="bp", bufs=8) as bp,
            tc.tile_pool(name="tp", bufs=4) as tp,
            tc.tile_pool(name="op", bufs=4) as op,
        ):
            xr = x.rearrange("h c b w -> h c (b w)")
            X = xp.tile([H, C * BPC * W], bf16)
            Xv = X.rearrange("p (c b w) -> p c b w", c=C, b=BPC)
            Xf = X.rearrange("p (c q) -> p c q", c=C)
            for ci in range(NXCHUNK):
                nc.sync.dma_start(
                    out=Xf[:, ci * CSZ : (ci + 1) * CSZ],
                    in_=xr[:, ci * CSZ : (ci + 1) * CSZ],
                )

            S = xp.tile([H, ncols * BPC * W], bf16)
            Sv = S.rearrange("p (j b w) -> p j b w", j=ncols, b=BPC)
            Sf = S.rearrange("p (j q) -> p j q", j=ncols)
            use_gp = any(ge == "gp" for _, _, ge in groups)
            Gc = xp.tile([H, K], f32) if use_gp else None
            with tc.high_priority(offset=1_000_000):
                # finite filler for exactly the junk rows of each run's
                # columns (head rows [0:-s] for s<0, tail [H-s:H] for s>0).
                # Never touches rows the run writes, so filler/run order is
                # free (high_priority flattens priorities; an overlapping
                # filler racing a run corrupted valid rows).
                for (s, c0, cl, d0) in runlist:
                    n = cl - c0 + 1
                    if s < 0:
                        nc.gpsimd.dma_start(
                            out=Sf[0:-s, d0 : d0 + n], in_=xr[0:-s, 0:n]
                        )
                    else:
                        nc.gpsimd.dma_start(
                            out=Sf[H - s : H, d0 : d0 + n], in_=xr[0:s, 0:n]
                        )
                for (s, c0, cl, d0) in runlist:
                    # S[p, d0+i] = x[c0+i, p+s].  Chunked to <=8 columns:
                    # per-partition descriptors >4KB sometimes hash onto a
                    # single SDMA engine (measured: a 22KB-descriptor run
                    # serialized at 27 GB/s for 106us); <=4KB always spreads.
                    for cc in range(c0, cl + 1, 8):
                        ce = min(cc + 8, cl + 1)
                        n = ce - cc
                        dd = d0 + (cc - c0)
                        if s < 0:
                            nc.gpsimd.dma_start(
                                out=Sf[-s:H, dd : dd + n],
                                in_=xr[0 : H + s, cc:ce],
                            )
                        else:
                            nc.gpsimd.dma_start(
                                out=Sf[0 : H - s, dd : dd + n],
                                in_=xr[s:H, cc:ce],
                            )
                if use_gp:
                    nc.gpsimd.dma_start(out=Gc, in_=gcd[:, :])

            out_kb = out.rearrange("oh k b ow -> oh (k b) ow")
            fd = BPC * OW

            def odd(ap):
                # [p, (4*63)] view: odd innermost dim caps DVE TS at 2x_1P so
                # it never grabs the DVE/GpSimd shared SBUF port pair.
                return ap.rearrange("p (a q) -> p a q", a=4)

            def emit_gamma_and_store(pos0, ks, geng, T, O):
                # deferred one group so cross-engine waits are pre-satisfied
                for j, k in enumerate(ks):
                    _, base, _, _, path, scal, gamma = plans[k]
                    cnt = base + OH
                    slot = T[0:cnt, j * fd : (j + 1) * fd]
                    osl = O[0:cnt, j * fd : (j + 1) * fd]
                    if geng == "act":
                        nc.scalar.activation(osl, slot, Copy, bias=gamma, scale=1.0)
                    elif geng == "dve":
                        nc.vector.tensor_scalar(odd(osl), odd(slot), gamma, None, add)
                    else:
                        gb = Gc[0:cnt, pos0 + j : pos0 + j + 1].broadcast_to([cnt, fd])
                        nc.gpsimd.tensor_tensor(osl, slot, gb, add)
                # batched stores per same-base run: SWDGE on the GpSimd queue
                i = 0
                while i < len(ks):
                    base = plans[ks[i]][1]
                    i2 = i
                    while i2 < len(ks) and plans[ks[i2]][1] == base:
                        i2 += 1
                    src = O[base : base + OH, i * fd : i2 * fd].rearrange(
                        "p (kb w) -> p kb w", w=OW
                    )
                    dst = out_kb[:, (pos0 + i) * BPC : (pos0 + i2) * BPC]
                    nc.gpsimd.dma_start(out=dst, in_=src)
                    i = i2

            pending = None
            pos0 = 0
            for gi, (ks, gdt, geng) in enumerate(groups):
                dt = bf16 if gdt == "bf" else f32
                T = tp.tile([H, GRP * fd], dt, tag="t", name=f"t_{gi}")
                O = op.tile([H, GRP * fd], f32, tag="o", name=f"o_{gi}")

                for j, k in enumerate(ks):
                    _, base, a_src, b_src, path, scal, gamma = plans[k]
                    cnt = base + OH

                    def view(src):
                        shifted, idx, woff = src
                        t = Sv if shifted else Xv
                        return t[0:cnt, idx, :, woff : woff + OW]

                    Av, Bv = view(a_src), view(b_src)
                    slot = T[0:cnt, j * fd : (j + 1) * fd]
                    slotv = slot.rearrange("p (b w) -> p b w", b=BPC)
                    b2 = bp.tile([H, fd], dt, tag="b2", name=f"b2_{k}")
                    b2v = b2.rearrange("p (b w) -> p b w", b=BPC)[0:cnt]

                    if path == "fact":
                        kab, kka, alpha = scal
                        nc.scalar.activation(b2v, Bv, Copy, bias=kka, scale=kab)
                        nc.vector.scalar_tensor_tensor(slotv, Av, alpha, b2v, add, mult)
                    else:  # linear/exact: slot = Ca*A + (Cb*B + C1)
                        if path == "linear":
                            kka, kkb, kk1 = scal
                        else:
                            kab, kka, kkb, kk1 = scal
                        nc.scalar.activation(b2v, Bv, Copy, bias=kk1, scale=kkb)
                        nc.vector.scalar_tensor_tensor(slotv, Av, kka, b2v, mult, add)
                        if path == "exact":  # += (Cab*B)*A
                            p2 = bp.tile([H, fd], dt, tag="b2", name=f"p2_{k}")
                            p2v = p2.rearrange("p (b w) -> p b w", b=BPC)[0:cnt]
                            nc.vector.scalar_tensor_tensor(p2v, Bv, kab, Av, mult, mult)
                            nc.vector.tensor_tensor(slot, slot, p2[0:cnt], add)

                if pending is not None:
                    emit_gamma_and_store(*pending)
                pending = (pos0, ks, geng, T, O)
                pos0 += len(ks)
            if pending is not None:
                emit_gamma_and_store(*pending)
    nc.compile()
    return nc


def kernel(x, pairs_a, pairs_b, weights):
    import ml_dtypes
    from concourse.bass_utils import run_bass_kernel_spmd

    x = np.ascontiguousarray(np.asarray(x), dtype=np.float32)
    pa = np.asarray(pairs_a).astype(np.int64)
    pb = np.asarray(pairs_b).astype(np.int64)
    w = np.asarray(weights).astype(np.float32)

    nc = _build(pa, pb, w, x[0])
    _, _, _, order, gcol = _plan(pa, pb, w, x[0])
    xq = x.astype(ml_dtypes.bfloat16)
    in_maps = [
        {
            "x": np.ascontiguousarray(
                xq[i * BPC : (i + 1) * BPC].transpose(2, 1, 0, 3)
            ),
            "gcol": gcol,
        }
        for i in range(NCORES)
    ]
    res = run_bass_kernel_spmd(nc, in_maps, core_ids=list(range(NCORES)))
    # device layout [OH, K(sorted), BPC, OW] per core -> [B, K, OH, OW]
    full = np.concatenate([r["out"] for r in res.results], axis=2)  # [OH,K,B,OW]
    pos = np.empty(K, np.int64)
    pos[np.asarray(order)] = np.arange(K)
    return np.ascontiguousarray(full[:, pos].transpose(2, 1, 0, 3))
